# revision 1
# baseline (speedup 1.0000x reference)
"""Differentiable Bezier path renderer on 8 Trainium2 NeuronCores.

Strategy
--------
The reference rasterizes M=2048 path edges into a 512x512 soft
winding-number image:

    wind[h, w] = sum_e coeff(e, h) * sigmoid(x_cross(e, h) - w)
    coeff(e,h) = sigmoid(20 t) * sigmoid(20 (1 - t)) * sign(dy_e) * [|dy_e|>=1e-6]
    t          = (h - y0_e) / (dy_e + 1e-8),  x_cross = x0_e + t * dx_e

Two sparsity facts carry the kernel:
  * coeff is negligible (< 5e-8) outside t in [-0.85, 1.85], so only
    ~55k of the 1M (edge, row) pairs matter.
  * sigmoid(x_cross - w) saturates outside |x_cross - w| <= 18, so per
    pair only a 36px-wide transition window needs real sigmoids; the
    whole region left of the window contributes exactly coeff.

The host enumerates active pairs, assigns rows to cores so every core
gets an equal pair load (64 rows per core, no collectives needed), and
packs pairs into blocks of 128 "slots".  Each pair gets one 128-wide,
64-aligned window segment s (columns [64 s, 64 s + 128)) that is
guaranteed to contain its transition; blocks group pairs of the same s.

Per block the device computes, slots on the partition axis:
  * ScalarE : SIGW[p, k] = sigmoid((xc_p - 64 s) - k), k in [0,128)
  * VectorE : W2[p, r] = (iota_r == row_p) * coeff_p     (fused is_eq*mul)
  * TensorE : PSW[r, s-slice] += W2.T @ SIGW             (window part)
              PSL[r, b]       += W2.T @ LMASK            (saturated part,
                LMASK[p, b] = [64 (b+1) <= 64 s_p], b in [0,7))
Afterwards VectorE folds the 8 overlapping stream slices plus the
broadcast left-constants into wind[64, 512], and ScalarE writes
alpha = sigmoid(4 wind) into an interleaved RGBA tile (rgb = broadcast
input color).  The host only gathers per-edge scalars per pair and
reassembles the 8 per-core row sets.
"""

import numpy as np

import concourse.bacc as bacc
import concourse.mybir as mybir
import concourse.tile as tile
from concourse.bass_utils import run_bass_kernel_spmd

H = 512
W = 512
S = 64          # cubic bezier segments
TSAMP = 32      # samples per segment
M = S * TSAMP   # path points == edges
NCORES = 8
RPC = H // NCORES  # rows per core
NSTREAM = 8        # 64-aligned window segment streams
WIN = 18.0         # sigmoid saturation half-width (sigmoid(-18) ~ 1.5e-8)
TB = np.float32(0.85)     # t-window bound: sigmoid(-17) ~ 4.1e-8
CLAMP_T = 60.0            # |20 t| <= 1200, keeps ACT inputs finite
CLAMP_X = 10000.0         # sigmoid saturated way before +-CLAMP_X
DT = mybir.dt.float32
AF = mybir.ActivationFunctionType
PNAMES = ("y0", "rc", "x0", "dx", "sm", "gy", "so", "rl")

_prog_cache = {}


def _host_prep(control_points):
    """Sample the path, enumerate active (edge, row) pairs, assign rows to
    cores, pack pairs into per-stream blocks of 128 slots.

    Returns (per_core_inputs, core_rows, stream_blocks) where
    stream_blocks[s] is the number of blocks of stream s (same for all
    cores; short cores are padded with coeff=0 slots)."""
    cp = np.asarray(control_points, dtype=np.float32)
    p0 = cp[0:3 * S:3][:, None, :]
    p1 = cp[1:3 * S:3][:, None, :]
    p2 = cp[2:3 * S:3][:, None, :]
    p3 = cp[3:3 * S + 1:3][:, None, :]
    t = np.linspace(0.0, 1.0, TSAMP, dtype=np.float32)[None, :, None]
    mt = np.float32(1.0) - t
    pts = (mt ** 3) * p0 + 3.0 * (mt ** 2) * t * p1 \
        + 3.0 * mt * (t ** 2) * p2 + (t ** 3) * p3
    path = pts.reshape(-1, 2).astype(np.float32)

    nxt = np.roll(path, -1, axis=0)
    x0 = path[:, 0]
    y0 = path[:, 1]
    dy = nxt[:, 1] - y0
    dxe = nxt[:, 0] - x0
    dys = (dy + np.float32(1e-8)).astype(np.float32)
    recip = (np.float32(1.0) / dys).astype(np.float32)
    sm = (np.sign(dy) * (np.abs(dy) >= np.float32(1e-6))).astype(np.float32)

    g1 = y0 + (-TB) * dys
    g2 = y0 + (np.float32(1.0) + TB) * dys
    rlo = np.maximum(np.ceil(np.minimum(g1, g2)), 0.0).astype(np.int64)
    rhi = np.minimum(np.floor(np.maximum(g1, g2)), H - 1).astype(np.int64)
    act = (sm != 0) & (rhi >= rlo)
    eact = np.nonzero(act)[0]
    counts = (rhi[eact] - rlo[eact] + 1).astype(np.int64)
    pair_edge = np.repeat(eact, counts)
    pair_row = np.concatenate(
        [np.arange(rlo[e], rhi[e] + 1, dtype=np.int64) for e in eact]
    ) if len(eact) else np.zeros(0, np.int64)

    # Window segment per pair, from host-side x_cross (the ~1 ulp
    # host/device difference is covered by the 64 - 36 px fit margin).
    tval = ((pair_row.astype(np.float32) - y0[pair_edge]) * recip[pair_edge])
    xcv = x0[pair_edge] + tval * dxe[pair_edge]
    xcv = np.clip(xcv, -CLAMP_X, CLAMP_X)
    seg = np.clip(np.floor((xcv - WIN) / 64.0), 0, NSTREAM - 1).astype(np.int64)

    # Balanced row -> core assignment (equal pair load, RPC rows per core).
    rowcnt = np.bincount(pair_row, minlength=H)
    order = np.argsort(-rowcnt, kind="stable")
    core_rows = [[] for _ in range(NCORES)]
    loads = np.zeros(NCORES, np.int64)
    for r in order:
        avail = [c for c in range(NCORES) if len(core_rows[c]) < RPC]
        c = min(avail, key=lambda i: loads[i])
        core_rows[c].append(int(r))
        loads[c] += rowcnt[r]
    row_core = np.empty(H, np.int64)
    row_loc = np.empty(H, np.int64)
    for c in range(NCORES):
        for i, r in enumerate(core_rows[c]):
            row_core[r] = c
            row_loc[r] = i

    pair_core = row_core[pair_row]
    # blocks per stream = max over cores (SPMD: one program for all cores),
    # rounded up so near-identical inputs reuse the compiled program.
    stream_blocks = []
    for s in range(NSTREAM):
        ns = np.array([((pair_core == c) & (seg == s)).sum()
                       for c in range(NCORES)])
        nb = max(1, int(np.ceil(ns.max() / 128.0)))
        stream_blocks.append(nb)
    total_nb = sum(stream_blocks)
    pad_round = int(np.ceil(total_nb / 8.0)) * 8 - total_nb
    stream_blocks[0] += pad_round  # round total to a multiple of 8

    NBT = sum(stream_blocks)
    per_core = []
    for c in range(NCORES):
        vals = {k: np.zeros(NBT * 128, np.float32) for k in PNAMES}
        off = 0
        for s in reversed(range(NSTREAM)):
            nb = stream_blocks[s]
            if nb == 0:
                continue
            idx = np.nonzero((pair_core == c) & (seg == s))[0]
            n = len(idx)
            sl = slice(off * 128, off * 128 + n)
            pe = pair_edge[idx]
            vals["y0"][sl] = y0[pe]
            vals["rc"][sl] = recip[pe]
            vals["x0"][sl] = x0[pe]
            vals["dx"][sl] = dxe[pe]
            vals["sm"][sl] = sm[pe]
            vals["gy"][sl] = pair_row[idx].astype(np.float32)
            vals["so"][sl] = np.float32(64.0) * s
            vals["rl"][sl] = row_loc[pair_row[idx]].astype(np.float32)
            off += nb
        packed = np.concatenate(
            [vals[k].reshape(NBT, 128).T for k in PNAMES] +
            [np.zeros((128, 4), np.float32)], axis=1)
        per_core.append({"params": np.ascontiguousarray(packed)})
    return per_core, core_rows, tuple(stream_blocks)


def _build_program(stream_blocks, repeats=1):
    key = (stream_blocks, repeats)
    if key in _prog_cache:
        return _prog_cache[key]
    NBT = sum(stream_blocks)
    nc = bacc.Bacc("TRN2", target_bir_lowering=False, debug=False,
                   num_devices=NCORES)

    npar = len(PNAMES) * NBT + 4
    pard = nc.dram_tensor("params", [128, npar], DT, kind="ExternalInput")
    outd = nc.dram_tensor("rgba", [RPC, W * 4], DT, kind="ExternalOutput")

    cst = np.zeros((128, 130 + RPC), np.float32)
    cst[:, :128] = np.arange(128, dtype=np.float32)[None, :]
    cst[:, 128:130] = -20000.0
    cst[:, 130:] = np.arange(RPC, dtype=np.float32)[None, :]
    cstd = nc.inline_tensor(np.ascontiguousarray(cst), name="cstconst")

    import contextlib

    with tile.TileContext(nc) as tc:
        with (
            tc.tile_pool(name="const", bufs=1) as cpool,
            tc.tile_pool(name="sig", bufs=4) as sigpool,
            tc.tile_pool(name="w2", bufs=4) as w2pool,
            tc.tile_pool(name="psum", bufs=1, space="PSUM") as pspool,
            (tc.For_i(0, repeats, 1) if repeats > 1
             else contextlib.nullcontext()),
        ):
            cstt = cpool.tile([128, 130 + RPC], DT)
            nc.sync.dma_start(cstt[:], cstd[:])
            k130t = cstt[:, 0:130]
            r64t = cstt[:, 130:130 + RPC]
            part = cpool.tile([128, npar], DT)
            nc.sync.dma_start(part[:], pard[:])
            cbt = part[0:RPC, len(PNAMES) * NBT:len(PNAMES) * NBT + 4]
            tin = {n: part[:, i * NBT:(i + 1) * NBT]
                   for i, n in enumerate(PNAMES)}

            # t = (gy - y0) * recip;  bias = clamp(x0 + t * dx) - so
            # coeff = sigmoid(20 t) * sigmoid(20 - 20 t) * sm
            # computed in column chunks so the first blocks unblock early
            b20 = cpool.tile([128, 1], DT)
            nc.vector.memset(b20[:], 20.0)
            tt = cpool.tile([128, NBT], DT)
            xct = cpool.tile([128, NBT], DT)
            tcl = cpool.tile([128, NBT], DT)
            v1 = cpool.tile([128, NBT], DT)
            v2 = cpool.tile([128, NBT], DT)
            cft = cpool.tile([128, NBT], DT)
            for c0 in [0]:
                ch = slice(0, NBT)
                nc.vector.tensor_sub(tt[:, ch], tin["gy"][:, ch],
                                     tin["y0"][:, ch])
                nc.vector.tensor_mul(tt[:, ch], tt[:, ch], tin["rc"][:, ch])
                nc.vector.tensor_mul(xct[:, ch], tt[:, ch], tin["dx"][:, ch])
                nc.vector.tensor_add(xct[:, ch], xct[:, ch], tin["x0"][:, ch])
                nc.vector.tensor_scalar_min(xct[:, ch], xct[:, ch], CLAMP_X)
                nc.vector.tensor_scalar_max(xct[:, ch], xct[:, ch], -CLAMP_X)
                nc.vector.tensor_sub(xct[:, ch], xct[:, ch], tin["so"][:, ch])
                nc.vector.tensor_scalar_min(tcl[:, ch], tt[:, ch], CLAMP_T)
                nc.vector.tensor_scalar_max(tcl[:, ch], tcl[:, ch], -CLAMP_T)
                nc.scalar.activation(v1[:, ch], tcl[:, ch], AF.Sigmoid,
                                     bias=0.0, scale=20.0)
                nc.scalar.activation(v2[:, ch], tcl[:, ch], AF.Sigmoid,
                                     bias=b20[:], scale=-20.0)
                nc.vector.tensor_mul(cft[:, ch], v1[:, ch], v2[:, ch])
                nc.vector.tensor_mul(cft[:, ch], cft[:, ch], tin["sm"][:, ch])

            rgba = cpool.tile([RPC, W * 4], DT)
            for ch in range(3):
                nc.vector.tensor_copy(
                    rgba[:, ch::4],
                    cbt[:, ch:ch + 1].broadcast_to((RPC, W)))
            rgba4 = rgba[:].rearrange("p (w c) -> p w c", c=4)

            # SW = 130-wide stream slices: 128 sigmoid cols + 2 saturated
            # (==1.0) cols whose matmul output is the stream's coeff row-sum.
            SW = 130
            pst = [pspool.tile([RPC, SW], DT, name=f"psw{s}", tag=f"psw{s}")
                   for s in range(NSTREAM)]
            wind = cpool.tile([RPC, W], DT)
            suf = cpool.tile([RPC, NSTREAM], DT)  # suf[:, b] = sum_{s>b} rowsum_s
            rev = list(reversed(range(NSTREAM)))
            jbase = {}
            acc = 0
            for s in rev:
                jbase[s] = acc
                acc += stream_blocks[s]
            for si, s in enumerate(rev):
                for js in range(stream_blocks[s]):
                    j = jbase[s] + js
                    w2 = w2pool.tile([128, RPC], DT)
                    nc.vector.tensor_scalar(
                        w2[:], r64t, tin["rl"][:, j:j + 1],
                        cft[:, j:j + 1], mybir.AluOpType.is_equal,
                        mybir.AluOpType.mult)
                    sig = sigpool.tile([128, SW], DT)
                    nc.scalar.activation(sig[:], k130t, AF.Sigmoid,
                                         bias=xct[:, j:j + 1], scale=-1.0)
                    nc.tensor.matmul(pst[s][:], w2[:],
                                     sig[:], start=(js == 0),
                                     stop=(js == stream_blocks[s] - 1))
                # stream s complete: extend suffix sums, fold ready blocks
                if si == 0:
                    nc.vector.memset(suf[:, s:s + 1], 0.0)
                else:
                    nc.vector.tensor_scalar_add(suf[:, s:s + 1],
                                                pst[s + 1][:, 128:129],
                                                suf[:, s + 1:s + 2])
                # col-block b = s + 1 needs streams s and s+1 (both done)
                if si > 0:
                    b = s + 1
                    dst = wind[:, b * 64:(b + 1) * 64]
                    nc.vector.tensor_scalar_add(dst, pst[s][:, 64:128],
                                                suf[:, b:b + 1])
                    nc.vector.tensor_add(dst, dst, pst[b][:, 0:64])
                if s == 0:
                    nc.vector.tensor_scalar_add(wind[:, 0:64],
                                                pst[0][:, 0:64], suf[:, 0:1])
                # alpha + output as soon as a 256-col half is folded
                if s == 3:
                    nc.scalar.activation(rgba4[:, 256:512, 3],
                                         wind[:, 256:512], AF.Sigmoid,
                                         bias=0.0, scale=4.0)
                    nc.sync.dma_start(outd[:, 1024:2048],
                                      rgba[:, 1024:2048])
                if s == 0:
                    nc.scalar.activation(rgba4[:, 0:256, 3],
                                         wind[:, 0:256], AF.Sigmoid,
                                         bias=0.0, scale=4.0)
                    nc.sync.dma_start(outd[:, 0:1024], rgba[:, 0:1024])

    nc.compile()
    _prog_cache[key] = nc
    return nc


def _in_maps(per_core, color):
    maps = []
    for c in range(NCORES):
        p = per_core[c]["params"].copy()
        p[:RPC, -4:-1] = np.asarray(color, np.float32)[None, :]
        maps.append({"params": p})
    return maps


def kernel(control_points, color):
    per_core, core_rows, stream_blocks = _host_prep(control_points)
    nc = _build_program(stream_blocks)
    res = run_bass_kernel_spmd(nc, _in_maps(per_core, color),
                               list(range(NCORES)))
    out = np.empty((H, W, 4), np.float32)
    for c in range(NCORES):
        rg = res.results[c]["rgba"].reshape(RPC, W, 4)
        out[np.asarray(core_rows[c], np.int64)] = rg
    return out



# revision 6
# speedup vs baseline: 2.1364x; 2.1364x over previous
"""Differentiable Bezier path renderer on 8 Trainium2 NeuronCores.

Strategy (v2)
-------------
The reference rasterizes M=2048 path edges into a 512x512 soft
winding-number image:

    wind[h, w] = sum_e coeff(e, h) * sigmoid(x_cross(e, h) - w)
    alpha      = sigmoid(4 * wind),  rgb = broadcast(color)

Only (edge, row) pairs with t in [-TB, 1+TB] matter (~40k of 1M), and
per pair only a ~22px sigmoid transition window needs evaluation; left
of the window the pair contributes exactly coeff, right of it zero.

The host enumerates active pairs, computes their two scalars (coeff,
window-relative x_cross), assigns rows to cores with equal pair load
(64 rows/core, no collectives), buckets pairs into 32px-aligned
streams s (transition inside cols [32s, 32s+56)), and packs blocks of
128 slots.  It ships, per core, fp16 tensors:
  * w2[p, j*64 + r] = coeff_p * [row_p == r]   (one-hot scatter matrix)
  * xcf[p, j]       = x_cross_p - 32*s_p       (fp32)

Device per block j (slots on partitions), engines pipelined:
  * DVE    : ARG[p, jk] = xcf[p,j] - k          (one batched op/group)
  * ScalarE: SIG = sigmoid(ARG)                 (one batched op/group)
  * TensorE: wind[r, 32s+k]  += w2_j.T @ SIG_j   (fp16, 1 cyc/col,
             psum accumulation at absolute columns; 4 quarter banks)
             LS[r, b]        += w2_j.T @ LMASK_s  (coarse 32px left sums)
Streams are processed right-to-left so each 128-col quarter finalizes
(VectorE adds the broadcast left-sums in psum, ScalarE writes
alpha = sigmoid(4 wind) to SBUF, DMA out) while matmuls continue.
The host assembles rgb = color and re-orders the 8 row sets.
"""

import numpy as np

import concourse.bacc as bacc
import concourse.mybir as mybir
import concourse.tile as tile
from concourse.bass_utils import run_bass_kernel_spmd

H = 512
W = 512
S = 64          # cubic bezier segments
TSAMP = 32      # samples per segment
M = S * TSAMP   # path points == edges
NCORES = 8
RPC = H // NCORES  # rows per core
NSTREAM = 16       # 32px-aligned window streams
A = 32             # stream alignment
SW = 56            # sigmoid window columns per pair
C = 11.0           # sigmoid saturation half-width (sigmoid(-11) ~ 1.7e-5)
TB = np.float32(0.6)   # t-window bound: sigmoid(-12) ~ 6.1e-6
DT = mybir.dt.float32
F16 = mybir.dt.float16
AF = mybir.ActivationFunctionType

_prog_cache = {}


def _sigmoid64(z):
    with np.errstate(over="ignore", under="ignore"):
        return 1.0 / (1.0 + np.exp(-z.astype(np.float64)))


def _host_prep(control_points):
    """Sample the path, enumerate active (edge, row) pairs, assign rows to
    cores, bucket pairs into streams, pack 128-slot blocks.

    Returns (per_core_inputs, core_rows, nbs) where nbs[s] is the block
    count of stream s (same for all cores; short cores padded with
    zero-coeff slots)."""
    cp = np.asarray(control_points, dtype=np.float32)
    p0 = cp[0:3 * S:3][:, None, :]
    p1 = cp[1:3 * S:3][:, None, :]
    p2 = cp[2:3 * S:3][:, None, :]
    p3 = cp[3:3 * S + 1:3][:, None, :]
    t = np.linspace(0.0, 1.0, TSAMP, dtype=np.float32)[None, :, None]
    mt = np.float32(1.0) - t
    pts = (mt ** 3) * p0 + 3.0 * (mt ** 2) * t * p1 \
        + 3.0 * mt * (t ** 2) * p2 + (t ** 3) * p3
    path = pts.reshape(-1, 2).astype(np.float32)

    nxt = np.roll(path, -1, axis=0)
    x0 = path[:, 0]
    y0 = path[:, 1]
    dy = nxt[:, 1] - y0
    dxe = nxt[:, 0] - x0
    dys = (dy + np.float32(1e-8)).astype(np.float32)
    recip = (np.float32(1.0) / dys).astype(np.float32)
    sm = (np.sign(dy) * (np.abs(dy) >= np.float32(1e-6))).astype(np.float32)

    g1 = y0 + (-TB) * dys
    g2 = y0 + (np.float32(1.0) + TB) * dys
    rlo = np.maximum(np.ceil(np.minimum(g1, g2)), 0.0).astype(np.int64)
    rhi = np.minimum(np.floor(np.maximum(g1, g2)), H - 1).astype(np.int64)
    act = (sm != 0) & (rhi >= rlo)
    eact = np.nonzero(act)[0]
    counts = (rhi[eact] - rlo[eact] + 1).astype(np.int64)
    pair_edge = np.repeat(eact, counts)
    pair_row = np.concatenate(
        [np.arange(rlo[e], rhi[e] + 1, dtype=np.int64) for e in eact]
    ) if len(eact) else np.zeros(0, np.int64)

    tval = ((pair_row.astype(np.float32) - y0[pair_edge]) * recip[pair_edge])
    cf = (_sigmoid64(20.0 * tval) * _sigmoid64(20.0 * (1.0 - tval))
          * sm[pair_edge]).astype(np.float32)
    xcv = (x0[pair_edge] + tval * dxe[pair_edge]).astype(np.float32)

    keep = xcv >= -C   # pairs entirely left of the image contribute ~0
    pair_row = pair_row[keep]
    cf = cf[keep]
    xcv = xcv[keep]

    seg = np.clip(np.floor((xcv - C) / A), 0, NSTREAM - 1).astype(np.int64)
    xcf = np.clip(xcv - A * seg.astype(np.float32), -60.0, 60.0)

    # Balanced row -> core assignment (equal pair load, RPC rows per core).
    rowcnt = np.bincount(pair_row, minlength=H)
    order = np.argsort(-rowcnt, kind="stable")
    core_rows = [[] for _ in range(NCORES)]
    loads = np.zeros(NCORES, np.int64)
    for r in order:
        avail = [c for c in range(NCORES) if len(core_rows[c]) < RPC]
        c = min(avail, key=lambda i: loads[i])
        core_rows[c].append(int(r))
        loads[c] += rowcnt[r]
    row_core = np.empty(H, np.int64)
    row_loc = np.empty(H, np.int64)
    for c in range(NCORES):
        for i, r in enumerate(core_rows[c]):
            row_core[r] = c
            row_loc[r] = i

    pair_core = row_core[pair_row]
    # blocks per stream = max over cores (SPMD: one program for all cores)
    nbs = []
    for s in range(NSTREAM):
        ns = np.array([((pair_core == c) & (seg == s)).sum()
                       for c in range(NCORES)])
        nbs.append(max(1, int(np.ceil(ns.max() / 128.0))))
    # round total block count to a multiple of 4 (compile-cache stability)
    total = sum(nbs)
    pad = (-total) % 4
    nbs[int(np.argmax(nbs))] += pad
    NBT = sum(nbs)

    rl_all = row_loc[pair_row]
    per_core = []
    for c in range(NCORES):
        w2 = np.zeros((128, NBT * 64), np.float16)
        xcfa = np.zeros((128, NBT), np.float32)
        j0 = 0
        for s in range(NSTREAM - 1, -1, -1):
            idx = np.nonzero((pair_core == c) & (seg == s))[0]
            m = np.arange(len(idx))
            b = j0 + m // 128
            p = m % 128
            w2[p, b * 64 + rl_all[idx]] = cf[idx].astype(np.float16)
            xcfa[p, b] = xcf[idx]
            j0 += nbs[s]
        per_core.append({"w2": w2, "xcf": xcfa})
    return per_core, core_rows, tuple(nbs)


def _build_program(nbs, repeats=1):
    key = (tuple(nbs), repeats)
    if key in _prog_cache:
        return _prog_cache[key]
    NBT = sum(nbs)
    nc = bacc.Bacc("TRN2", target_bir_lowering=False, debug=False,
                   num_devices=NCORES)

    w2d = nc.dram_tensor("w2", [128, NBT * 64], F16, kind="ExternalInput")
    xcfd = nc.dram_tensor("xcf", [128, NBT], DT, kind="ExternalInput")
    outd = nc.dram_tensor("alpha", [RPC, W], DT, kind="ExternalOutput")

    negk_np = np.tile(-np.arange(SW, dtype=np.float16)[None, :], (128, 1))
    negkd = nc.inline_tensor(np.ascontiguousarray(negk_np), name="negk")
    lmc_np = np.zeros((128, NSTREAM * 16), np.float16)
    for s in range(NSTREAM):
        for b in range(16):
            if b < s:
                lmc_np[:, s * 16 + b] = 1.0
    lmcd = nc.inline_tensor(np.ascontiguousarray(lmc_np), name="lmc")

    # processing order: streams right-to-left
    bl = []  # (j, stream, first_of_stream)
    j = 0
    for s in range(NSTREAM - 1, -1, -1):
        for i in range(nbs[s]):
            bl.append((j, s, i == 0))
            j += 1
    # stream completion -> quarter finalize triggers
    fin_after = {11: 3, 7: 2, 3: 1, 0: 0}

    # groups of blocks sharing one batched ARG + SIG instruction
    groups = []
    i = 0
    first_sz = 6
    while i < NBT:
        sz = first_sz if i == 0 else 10
        groups.append(bl[i:i + sz])
        i += sz

    import contextlib

    with tile.TileContext(nc) as tc:
        with (
            tc.tile_pool(name="const", bufs=1) as cpool,
            tc.tile_pool(name="w2p", bufs=3) as w2pool,
            tc.tile_pool(name="argp", bufs=3) as argpool,
            tc.tile_pool(name="sigp", bufs=3) as sigpool,
            tc.tile_pool(name="psum", bufs=1, space="PSUM") as pspool,
            (tc.For_i(0, repeats, 1) if repeats > 1
             else contextlib.nullcontext()),
        ):
            negkt = cpool.tile([128, SW], F16)
            nc.sync.dma_start(negkt[:], negkd[:])
            lmct = cpool.tile([128, NSTREAM * 16], F16)
            nc.sync.dma_start(lmct[:], lmcd[:])
            xcft = cpool.tile([128, NBT], DT)
            nc.sync.dma_start(xcft[:], xcfd[:])
            outt = cpool.tile([RPC, W], DT)

            wind = [pspool.tile([RPC, 128], DT, name=f"wind{q}",
                                tag=f"wind{q}") for q in range(4)]
            lsq = [pspool.tile([RPC, 4], DT, name=f"ls{q}", tag=f"ls{q}")
                   for q in range(4)]
            # PSUM has_written semantics: the first matmul per bank runs
            # start=True (clears the whole bank's bits); every later matmul
            # start=False accumulates where written, overwrites fresh cells.
            ls_started = [False] * 4
            wq_started = [False] * 4

            def fin(q):
                wq = wind[q]
                lss = cpool.tile([RPC, 4], DT, name=f"lss{q}", tag=f"lss{q}")
                nc.vector.tensor_copy(lss[:], lsq[q][:])
                nc.vector.tensor_tensor(
                    out=wq[:].rearrange("p (b k) -> p b k", k=32),
                    in0=wq[:].rearrange("p (b k) -> p b k", k=32),
                    in1=lss[:].unsqueeze(2).broadcast_to((RPC, 4, 32)),
                    op=mybir.AluOpType.add)
                nc.scalar.activation(outt[:, 128 * q:128 * (q + 1)], wq[:],
                                     AF.Sigmoid, bias=0.0, scale=4.0)
                nc.sync.dma_start(outd[:, 128 * q:128 * (q + 1)],
                                  outt[:, 128 * q:128 * (q + 1)])

            for gbl in groups:
                glen = len(gbl)
                j0 = gbl[0][0]
                w2t = w2pool.tile([128, glen * 64], F16, tag="w2")
                nc.sync.dma_start(w2t[:], w2d[:, j0 * 64:(j0 + glen) * 64])
                argt = argpool.tile([128, glen * SW], F16, tag="arg")
                nc.vector.tensor_tensor(
                    out=argt[:].rearrange("p (j k) -> p j k", k=SW),
                    in0=xcft[:, j0:j0 + glen].unsqueeze(2)
                        .broadcast_to((128, glen, SW)),
                    in1=negkt[:].unsqueeze(1).broadcast_to((128, glen, SW)),
                    op=mybir.AluOpType.add)
                sigt = sigpool.tile([128, glen * SW], F16, tag="sig")
                nc.scalar.activation(sigt[:], argt[:], AF.Sigmoid,
                                     bias=0.0, scale=1.0)

                for (jb, s, first) in gbl:
                    jj = jb - j0
                    lhsT = w2t[:, jj * 64:(jj + 1) * 64]
                    base = A * s
                    hi = min(base + SW, W)
                    c0 = base
                    while c0 < hi:
                        cq = min(hi, (c0 // 128 + 1) * 128)
                        q = c0 // 128
                        nc.tensor.matmul(
                            wind[q][:, c0 - 128 * q:cq - 128 * q], lhsT,
                            sigt[:, jj * SW + (c0 - base):
                                 jj * SW + (cq - base)],
                            start=(not wq_started[q]), stop=True,
                            skip_group_check=True)
                        wq_started[q] = True
                        c0 = cq
                    for qq in range(4):
                        if (not ls_started[qq]) or (4 * qq < s):
                            nc.tensor.matmul(
                                lsq[qq][:], lhsT,
                                lmct[:, s * 16 + 4 * qq:s * 16 + 4 * qq + 4],
                                start=(not ls_started[qq]), stop=True,
                                skip_group_check=True)
                            ls_started[qq] = True
                    # stream complete? (last block of stream s)
                    nj = jb + 1
                    nxt_s = None
                    for (j2, s2, _f2) in bl:
                        if j2 == nj:
                            nxt_s = s2
                            break
                    if (nj == NBT or (nxt_s is not None and nxt_s != s)) \
                            and s in fin_after:
                        fin(fin_after[s])

    nc.compile()
    _prog_cache[key] = nc
    return nc


def _in_maps(per_core, color):
    del color  # rgb assembled host-side
    return [{"w2": per_core[c]["w2"], "xcf": per_core[c]["xcf"]}
            for c in range(NCORES)]


def kernel(control_points, color):
    per_core, core_rows, nbs = _host_prep(control_points)
    nc = _build_program(nbs)
    res = run_bass_kernel_spmd(nc, _in_maps(per_core, color),
                               list(range(NCORES)))
    out = np.empty((H, W, 4), np.float32)
    out[:, :, :3] = np.asarray(color, np.float32)[None, None, :]
    for c in range(NCORES):
        out[np.asarray(core_rows[c], np.int64), :, 3] = \
            res.results[c]["alpha"]
    return out
